# revision 13
# baseline (speedup 1.0000x reference)
"""AutoCorrelation (channel-mean circular cross-correlation + rank-matched
delay aggregation) on 8 NeuronCores, pure data parallel over batch.

Math (per batch b, channels c = (h,e), C = 512, L = 512):
  mv[tau]  = (1/C) sum_c sum_t q_c[t] k_c[(t-tau) % L]    (= mean irfft(Q conj K))
  rank0    = descending ranks of mv[batch 0]
  g[b, j]  = softmax(mv[b])_sorted[ rank0[j] ]            (rank-matched scatter)
  out[b,t,c] = sum_u g[b,u] v[b,(t+u) % L, c]             (circular correlation)

Key change vs the FFT formulation: mv is computed WITHOUT any FFT.
  M'[t,s] = sum_c k[t,c] q[s,c]  (one 512^3 bf16 matmul per batch — half the
            PE work of the two DFT matmuls, and no DVE spectra products)
  mv[tau] = (1/C) sum_t M'[t, (t+tau) % L]
The diagonal sum is done by accumulating the four 128-row blocks of M' into
one PSUM tile with per-block column rotations (free: column-sliced matmul
outputs), round-tripping the folded P[128,512] through DRAM with a
(row-stride+1) diagonal read, and one ones-column matvec.

delays[0] comes from batch 0 only: instead of a collective (15us fixed cost
in the cost model), every core redundantly computes batch-0's M' block from
a replicated q0/k0 copy (3.4us PE, fully overlapped).

The delay-aggregation circulant is block-circulant with only FOUR distinct
[128,128] stationary blocks, so the stage-C stationary read is [128,512]
(128KB) instead of [128,2048], from a broadcast doubled-g row in DRAM via a
stride-1023 diagonal AP.

Rank codes use a single cnt_gt-256 encoding, exact in bf16 (|code| <= 256),
so the 16 equality masks run in DVE 2x/4x mode and the masks/g matmuls move
bf16. Emission order is the schedule (per-engine queues are in-order): PE
runs warmup -> M-blocks -> rank matvecs -> g matvecs -> stage C with
measured-readiness interleave so it never idles after ramp-up.
"""

import sys
for _p in ('/opt/trn_rl_repo',):
    if _p not in sys.path:
        sys.path.insert(0, _p)

import numpy as np
import ml_dtypes
from contextlib import ExitStack

import concourse.bass as bass
import concourse.bacc as bacc
import concourse.tile as tile
import concourse.mybir as mybir
from concourse.bass_utils import run_bass_kernel_spmd

F32 = mybir.dt.float32
F32R = mybir.dt.float32r
BF16 = mybir.dt.bfloat16
AL = mybir.AluOpType
AF = mybir.ActivationFunctionType
BF = ml_dtypes.bfloat16

B, L, H, E = 32, 512, 8, 64
C = H * E          # 512 channels per batch
NCORES = 8
NB = B // NCORES   # 4 local batches per core

N_WARM = 24        # PE warmup matmuls (p-state ramp while first load lands)


def _consts():
    # packed small consts: [antiI | two | one] -> [128, 130] f32
    small = np.zeros((128, 130), np.float32)
    small[np.arange(128), 127 - np.arange(128)] = 1.0   # anti-identity
    small[:, 128] = 2.0
    small[0, 129] = 1.0
    invc = np.full((128, 1), 1.0 / C, dtype=BF)         # 2^-9, exact in bf16
    return small, invc


_NC_CACHE = None
PE_LABELS = []


def _label_matmuls(nc):
    real = nc.tensor.matmul
    def wrapped(*a, **kw):
        import traceback
        frames = traceback.extract_stack()
        lab = "?"
        for fr in reversed(frames):
            if fr.filename.endswith("kernel.py") and fr.name != "wrapped":
                lab = f"{fr.name}:{fr.lineno}"
                break
        PE_LABELS.append(lab)
        return real(*a, **kw)
    nc.tensor.matmul = wrapped


def _build():
    global _NC_CACHE
    if _NC_CACHE is not None:
        return _NC_CACHE
    small_np, invc_np = _consts()

    nc = bacc.Bacc("TRN2", target_bir_lowering=False, debug=False, num_devices=NCORES)
    _label_matmuls(nc)
    tc = tile.TileContext(nc)

    # qk packed [slot, cchunk(4), {q,k}(2), p(128), t(L)] with q/k transposed
    # to [channel, time] on host so channels are the matmul contraction dim.
    qk_all = nc.dram_tensor("qk_all", [NB, 4, 2, 128, L], BF16, kind="ExternalInput")
    qk0_all = nc.dram_tensor("qk0_all", [4, 2, 128, L], BF16, kind="ExternalInput")
    v_all = nc.dram_tensor("v_all", [NB, L, C], BF16, kind="ExternalInput")
    out_all = nc.dram_tensor("out_all", [NB, L, C], BF16, kind="ExternalOutput")

    small_d = nc.inline_tensor(small_np, "small_d")
    invc_d = nc.inline_tensor(invc_np, "invc_d")

    with tc, ExitStack() as ctx:
        cpool = ctx.enter_context(tc.tile_pool(name="consts", bufs=1))
        qpool = ctx.enter_context(tc.tile_pool(name="qk", bufs=1))
        vpool = ctx.enter_context(tc.tile_pool(name="vv", bufs=1))
        wpool = ctx.enter_context(tc.tile_pool(name="work", bufs=1))
        pM = ctx.enter_context(tc.tile_pool(name="pM", bufs=1, space="PSUM"))
        pC = ctx.enter_context(tc.tile_pool(name="pC", bufs=1, space="PSUM"))
        psm = ctx.enter_context(tc.tile_pool(name="psm", bufs=1, space="PSUM"))
        dpool = ctx.enter_context(tc.tile_pool(name="dscratch", bufs=1, space="DRAM"))

        # ---- constants ----
        small_t = cpool.tile([128, 130], F32, name="small_t")
        antiI_t = small_t[:, 0:128]
        two_t = small_t[:, 128:129].bitcast(F32R)
        one_t = small_t[0:1, 129:130]
        invc_t = cpool.tile([128, 1], BF16, name="invc_t")

        # ---- PE warmup: ramp the p-state while the first loads land ----
        # (junk tile is never written: no input dependency, data irrelevant;
        #  output goes into the Mps ring; never read, reset by M0's start=True)
        junk_t = cpool.tile([128, 128], BF16, name="junk_t")
        nc.vector.memset(junk_t[:], 0.0)
        warm_ps = pM.tile([128, 512], F32, name="warm_ps", tag="Mps", bufs=2)
        for w in range(N_WARM):
            nc.tensor.matmul(warm_ps[:, 0:128], junk_t[:], junk_t[:],
                             start=True, stop=True)

        # ---- loads ----
        def load_qk_half(dram, base_off, qksb, h):
            nc.sync.dma_start(
                qksb[:, 2048 * h:2048 * (h + 1)],
                bass.AP(tensor=dram, offset=base_off + h * 2 * 2 * 128 * L,
                        ap=[[L, 128], [128 * L, 4], [1, L]]))

        def load_v(s):
            vsb = vpool.tile([128, 2048], BF16, name=f"v_s{s}", tag="vt", bufs=4)
            nc.sync.dma_start(vsb[:], bass.AP(tensor=v_all, offset=s * L * C,
                                              ap=[[C, 128], [128 * C, 4], [1, C]]))
            return vsb

        # ---- M' = k q^T with rotated fold into P (one PSUM tile) ----
        # M'[t,s] = sum_c k[t,c] q[s,c];  P[p,u] = sum_i M'[128i+p, (u+128i)%512]
        def emit_M(qksb, nm, ccs=(0, 1, 2, 3), M_ps=None):
            if M_ps is None:
                M_ps = pM.tile([128, 512], F32, name=f"M_{nm}", tag="Mps", bufs=2)
            for cc in ccs:
                qb = 1024 * cc
                kb = 1024 * cc + 512
                for i in range(4):
                    lhs = qksb[:, kb + 128 * i: kb + 128 * i + 128]
                    first = (cc == 0 and i == 0)
                    last = (cc == 3 and i == 3)
                    if i == 0:
                        nc.tensor.matmul(M_ps[:, 0:512], lhs, qksb[:, qb:qb + 512],
                                         start=first, stop=False, skip_group_check=True)
                    else:
                        w = 512 - 128 * i
                        nc.tensor.matmul(M_ps[:, 0:w], lhs, qksb[:, qb + 128 * i:qb + 512],
                                         start=False, stop=last, skip_group_check=True)
                        nc.tensor.matmul(M_ps[:, w:512], lhs, qksb[:, qb:qb + 128 * i],
                                         start=False, stop=last, skip_group_check=True)
            return M_ps

        # ---- P diag round trip: copies, one 640-wide write, shifted read ----
        def emit_Pdiag_w(M_ps, nm, q=None):
            q = q or nc.scalar
            P_sb = wpool.tile([128, 640], BF16, name=f"P_{nm}", tag="Psb", bufs=2)
            nc.scalar.copy(P_sb[:, 0:512], M_ps[:])
            nc.vector.tensor_copy(P_sb[:, 512:640], M_ps[:, 0:128])
            P_d = dpool.tile([128, 640], BF16, name=f"Pd_{nm}", tag="Pd", bufs=2)
            pd = P_d[:].tensor
            q.dma_start(bass.AP(tensor=pd, offset=0, ap=[[640, 128], [1, 640]]),
                        P_sb[:])
            return pd

        def emit_Pdiag_r(pd, nm, q=None):
            q = q or nc.scalar
            R_sb = wpool.tile([128, 512], BF16, name=f"R_{nm}", tag="Rsb", bufs=2)
            q.dma_start(R_sb[:], bass.AP(tensor=pd, offset=0, ap=[[641, 128], [1, 512]]))
            return R_sb

        def emit_Pdiag(M_ps, nm, q=None):
            return emit_Pdiag_r(emit_Pdiag_w(M_ps, nm, q), nm, q)

        # ---- mv[tau] = (1/C) sum_p R[p, tau] ----
        def emit_mv(R_sb, nm):
            mv_ps = psm.tile([1, 512], F32, name=f"mvps_{nm}", tag="psm", bufs=2)
            nc.tensor.matmul(mv_ps[:], invc_t[:], R_sb[:], start=True, stop=True)
            mv_sb = wpool.tile([1, 512], F32, name=f"mvsb_{nm}", tag="mvsb", bufs=5)
            nc.vector.tensor_copy(mv_sb[:], mv_ps[:])
            return mv_sb

        def emit_mvT(mv_sb, nm):
            mvT_ps = psm.tile([128, 4], F32, name=f"mvTps_{nm}", tag="psT", bufs=2)
            for j in range(4):
                nc.tensor.transpose(mvT_ps[:, j:j + 1], mv_sb[0:1, 128 * j:128 * (j + 1)], one_t)
            mvT_sb = wpool.tile([128, 4], F32, name=f"mvT_{nm}", tag="mvT", bufs=5)
            nc.scalar.copy(mvT_sb[:], mvT_ps[:])
            return mvT_sb

        # ---- per-slot rank codes (cnt_gt - 256 encoding, all on DVE) ----
        def finish_rank(s, mv_sb, mvT_sb):
            mvB = wpool.tile([128, 512], F32, name=f"mvB_{s}", tag="mvB", bufs=2)
            nc.gpsimd.partition_broadcast(mvB[:], mv_sb[:])
            rs = wpool.tile([128, 4], F32, name=f"rs_{s}", tag="rs", bufs=4)
            sgnscr = wpool.tile([128, 512], F32, name=f"sgn_{s}", tag="sgn", bufs=2)
            for j in range(4):
                nc.vector.tensor_scalar(sgnscr[:], mvB[:], mvT_sb[:, j:j + 1], None,
                                        AL.is_gt, AL.add, accum_out=rs[:, j:j + 1])
            # cnt_gt-256: bf16-exact integer in [-256,255]
            rsa = wpool.tile([128, 4], F32, name=f"rsa_{s}", tag="rsa", bufs=4)
            nc.vector.tensor_scalar(rsa[:], rs[:], -256.0, None, AL.add)
            rank_res[s] = rsa

        def finish_soft(s, mv_sb, mvT_sb):
            expz = wpool.tile([1, 512], F32, name=f"expz_{s}", tag="expz", bufs=2)
            z_sb = wpool.tile([1, 1], F32, name=f"z_{s}", tag="z", bufs=4)
            nc.scalar.activation(expz[:], mv_sb[:], AF.Exp, accum_out=z_sb[:])
            rz = wpool.tile([1, 1], F32, name=f"rz_{s}", tag="rz", bufs=4)
            nc.vector.reciprocal(rz[:], z_sb[:])
            smc = wpool.tile([128, 4], BF16, name=f"smc_{s}", tag="smc", bufs=4)
            nc.scalar.activation(smc[:], mvT_sb[:], AF.Exp)
            soft_res[s] = (rz, smc)

        def emit_wt(s):
            rsa = rank_res[s]
            wts = []
            for j in range(4):
                wt = wpool.tile([128, 512], BF16, name=f"wt_{s}_{j}", tag=f"wt{j}", bufs=2)
                nc.vector.tensor_scalar(wt[:], n2bB[:], rsa[:, j:j + 1], None, AL.is_equal)
                wts.append(wt)
            wt_res[s] = wts

        # ---- g row -> broadcast -> gmat tail -> 4-block circulant read ----
        def emit_gchain(s):
            rz, smc = soft_res[s]
            wts = wt_res[s]
            g_ps = psm.tile([1, 512], F32, name=f"gps_{s}", tag="psm", bufs=2)
            for j in range(4):
                nc.tensor.matmul(g_ps[:], smc[:, j:j + 1], wts[j][:], start=(j == 0), stop=(j == 3))
            gn = wpool.tile([1, 512], BF16, name=f"gn_{s}", tag="gn", bufs=2)
            nc.vector.tensor_scalar(gn[:], g_ps[:], rz[:], None, AL.mult)
            # gRB cols [128:640] = g broadcast; [0:128] = g[384:512] tail so the
            # doubled-row window [384,1024) of gmat is one contiguous write
            gRB = wpool.tile([128, 640], BF16, name=f"gRB_{s}", tag="gRB", bufs=2)
            nc.gpsimd.partition_broadcast(gRB[:, 128:640], gn[:])
            nc.vector.tensor_copy(gRB[:, 0:128], gRB[:, 512:640])
            gmat = dpool.tile([128, 1024], BF16, name=f"gmat_{s}", tag="gmat", bufs=2)
            gd = gmat[:].tensor
            nc.scalar.dma_start(bass.AP(tensor=gd, offset=384, ap=[[1024, 128], [1, 640]]),
                                gRB[:])
            cg4 = wpool.tile([128, 512], BF16, name=f"cg4_{s}", tag="cg4", bufs=2)
            nc.scalar.dma_start(cg4[:], bass.AP(tensor=gd, offset=511, ap=[[1023, 128], [1, 512]]))
            chain_res[s] = cg4

        # ---- stage C: block-circulant matmul, 4 distinct stationary blocks ----
        def emit_stagec(s, vsb, split_out=False):
            cg4 = chain_res[s]
            o_sb = wpool.tile([128, 2048], BF16, name=f"osb_{s}", tag="osb", bufs=2)
            for tt in range(4):
                o_ps = pC.tile([128, 512], F32, name=f"ops_{s}_{tt}", tag="ops", bufs=2)
                for ss in range(4):
                    m = (tt - ss) % 4
                    nc.tensor.matmul(o_ps[:], cg4[:, 128 * m:128 * (m + 1)],
                                     vsb[:, 512 * ss:512 * (ss + 1)],
                                     start=(ss == 0), stop=(ss == 3))
                if tt % 2 == 0:
                    nc.scalar.copy(o_sb[:, 512 * tt:512 * (tt + 1)], o_ps[:])
                else:
                    nc.vector.tensor_copy(o_sb[:, 512 * tt:512 * (tt + 1)], o_ps[:])
                if split_out:
                    nc.sync.dma_start(
                        bass.AP(tensor=out_all, offset=s * L * C + 128 * tt * C,
                                ap=[[C, 128], [1, C]]),
                        o_sb[:, 512 * tt:512 * (tt + 1)])
            if not split_out:
                nc.sync.dma_start(
                    bass.AP(tensor=out_all, offset=s * L * C,
                            ap=[[C, 128], [128 * C, 4], [1, C]]),
                    o_sb[:])

        rank_res, soft_res, wt_res, chain_res = {}, {}, {}, {}

        # ================= emission schedule =================
        # Emission order IS the per-engine execution order; all timing notes
        # from sim traces. The single DMA device is the latency conduit: the
        # SP queue carries the critical qk loads AND batch-0's P-diag round
        # trip (so loads cannot jump ahead of it); slot P-diags and g-chains
        # go on the ACT queue; v loads fill device holes.
        qk0sb = qpool.tile([128, 4096], BF16, name="qk0sb", tag="qkt", bufs=3)
        qksbs = [qpool.tile([128, 4096], BF16, name=f"qksb_{s}", tag="qkt", bufs=3)
                 for s in range(NB)]

        nc.sync.dma_start(invc_t[:], invc_d.ap())
        load_qk_half(qk0_all, 0, qk0sb, 0)
        nc.sync.dma_start(small_t[:], small_d.ap())
        load_qk_half(qk0_all, 0, qk0sb, 1)
        load_qk_half(qk_all, 0 * 1024 * L, qksbs[0], 0)
        load_qk_half(qk_all, 0 * 1024 * L, qksbs[0], 1)
        load_qk_half(qk_all, 1 * 1024 * L, qksbs[1], 0)
        load_qk_half(qk_all, 1 * 1024 * L, qksbs[1], 1)

        M0 = emit_M(qk0sb, "b0", ccs=(0, 1))
        emit_M(qk0sb, "b0", ccs=(2, 3), M_ps=M0)
        pd0 = emit_Pdiag_w(M0, "b0", q=nc.sync)   # SP: loads queue behind this
        vsbs = [load_v(0)]
        R0 = emit_Pdiag_r(pd0, "b0", q=nc.sync)
        load_qk_half(qk_all, 2 * 1024 * L, qksbs[2], 0)
        load_qk_half(qk_all, 2 * 1024 * L, qksbs[2], 1)

        Ms = [None] * NB
        R_s = [None] * NB
        Ms[0] = emit_M(qksbs[0], "s0")
        R_s[0] = emit_Pdiag(Ms[0], "s0")

        Ms[1] = emit_M(qksbs[1], "s1")
        mv0_sb = emit_mv(R0, "b0")
        posmvT = emit_mvT(mv0_sb, "b0")
        load_qk_half(qk_all, 3 * 1024 * L, qksbs[3], 0)
        load_qk_half(qk_all, 3 * 1024 * L, qksbs[3], 1)
        R_s[1] = emit_Pdiag(Ms[1], "s1")

        Ms[2] = emit_M(qksbs[2], "s2", ccs=(0, 1))
        # batch-0 reversal: mv0r[u] = mv0[511-u]
        mv0r_ps = psm.tile([1, 512], F32, name="mv0r_ps", tag="psm", bufs=2)
        for j in range(4):
            nc.tensor.matmul(mv0r_ps[0:1, 128 * (3 - j):128 * (4 - j)],
                             posmvT[:, j:j + 1], antiI_t, start=True, stop=True)
        mv0r_sb = wpool.tile([1, 512], F32, name="mv0r_sb", bufs=1)
        nc.scalar.copy(mv0r_sb[:], mv0r_ps[:])
        mvB0R = wpool.tile([128, 512], F32, name="mvB0R", bufs=1)
        nc.gpsimd.partition_broadcast(mvB0R[:], mv0r_sb[:])
        emit_M(qksbs[2], "s2", ccs=(2, 3), M_ps=Ms[2])

        # batch-0 rank counts: r2[u] = 2*cnt_gt0(rev u)
        r2_ps = psm.tile([1, 512], F32, name="r2_ps", tag="psm", bufs=2)
        for j in range(4):
            c2 = wpool.tile([128, 512], F32R, name=f"c2_{j}", tag="c2", bufs=2)
            nc.gpsimd.tensor_scalar(c2[:], mvB0R[:], posmvT[:, j:j + 1], None, AL.is_lt)
            nc.tensor.matmul(r2_ps[:], two_t, c2[:], start=(j == 0), stop=(j == 3))
        R_s[2] = emit_Pdiag(Ms[2], "s2")

        Ms[3] = emit_M(qksbs[3], "s3", ccs=(0, 1))
        # batch-0 per-position code row: cnt_gt0 - 256 (bf16-exact)
        n2bb_row = wpool.tile([1, 512], BF16, name="n2bb_row", bufs=1)
        nc.scalar.activation(n2bb_row[:], r2_ps[:], AF.Copy, bias=-256.0, scale=0.5)
        n2bB = wpool.tile([128, 512], BF16, name="n2bB", bufs=1)
        nc.gpsimd.partition_broadcast(n2bB[:], n2bb_row[:])
        mv_sb0 = emit_mv(R_s[0], "s0")
        mvT_sb0 = emit_mvT(mv_sb0, "s0")
        finish_rank(0, mv_sb0, mvT_sb0)
        emit_wt(0)
        emit_M(qksbs[3], "s3", ccs=(2, 3), M_ps=Ms[3])
        finish_soft(0, mv_sb0, mvT_sb0)
        R_s[3] = emit_Pdiag(Ms[3], "s3")
        vsbs.append(load_v(1))

        emit_gchain(0)
        mv_sb1 = emit_mv(R_s[1], "s1")
        mvT_sb1 = emit_mvT(mv_sb1, "s1")
        finish_rank(1, mv_sb1, mvT_sb1)
        emit_wt(1)
        finish_soft(1, mv_sb1, mvT_sb1)
        vsbs.append(load_v(2))

        mv_sb2 = emit_mv(R_s[2], "s2")
        mvT_sb2 = emit_mvT(mv_sb2, "s2")
        emit_gchain(1)
        finish_rank(2, mv_sb2, mvT_sb2)
        emit_wt(2)
        finish_soft(2, mv_sb2, mvT_sb2)
        vsbs.append(load_v(3))

        emit_stagec(0, vsbs[0])
        mv_sb3 = emit_mv(R_s[3], "s3")
        mvT_sb3 = emit_mvT(mv_sb3, "s3")
        emit_gchain(2)
        finish_rank(3, mv_sb3, mvT_sb3)
        emit_wt(3)
        finish_soft(3, mv_sb3, mvT_sb3)
        emit_stagec(1, vsbs[1])
        emit_gchain(3)
        emit_stagec(2, vsbs[2])
        emit_stagec(3, vsbs[3], split_out=True)

    nc.compile()
    _NC_CACHE = nc
    return nc


def kernel(queries, keys, values):
    q = np.ascontiguousarray(queries, dtype=np.float32).reshape(B, L, C)
    k = np.ascontiguousarray(keys, dtype=np.float32).reshape(B, L, C)
    v = np.ascontiguousarray(values, dtype=np.float32).reshape(B, L, C).astype(BF)
    # [B, C, L] -> [B, cchunk, {q,k}, 128, L]
    qT = np.ascontiguousarray(q.transpose(0, 2, 1)).astype(BF).reshape(B, 4, 128, L)
    kT = np.ascontiguousarray(k.transpose(0, 2, 1)).astype(BF).reshape(B, 4, 128, L)
    qk = np.stack([qT, kT], axis=2)  # [B, 4, 2, 128, L]
    nc = _build()
    in_maps = []
    for c in range(NCORES):
        sl = slice(NB * c, NB * (c + 1))
        in_maps.append({
            "qk_all": qk[sl],
            "qk0_all": qk[0],
            "v_all": v[sl],
        })
    res = run_bass_kernel_spmd(nc, in_maps, core_ids=list(range(NCORES)))
    out = np.concatenate([np.asarray(res.results[c]["out_all"]).astype(np.float32)
                          for c in range(NCORES)], axis=0)
    return out.reshape(B, L, H, E)


if __name__ == "__main__":
    rng = np.random.default_rng(0)
    qq = rng.standard_normal((B, L, H, E)).astype(np.float32)
    kk = rng.standard_normal((B, L, H, E)).astype(np.float32)
    vv = rng.standard_normal((B, L, H, E)).astype(np.float32)
    o = kernel(queries=qq, keys=kk, values=vv)
    print(o.shape, o.dtype, np.abs(o).max())


# revision 23
# speedup vs baseline: 1.1921x; 1.1921x over previous
"""AutoCorrelation (channel-mean circular cross-correlation + rank-matched
delay aggregation) on 8 NeuronCores, pure data parallel over batch.

Math (per batch b, channels c = (h,e), C = 512, L = 512):
  mv[tau]  = (1/C) sum_c sum_t q_c[t] k_c[(t-tau) % L]    (= mean irfft(Q conj K))
  rank0    = descending ranks of mv[batch 0]
  g[b, j]  = softmax(mv[b])_sorted[ rank0[j] ]            (rank-matched scatter)
  out[b,t,c] = sum_u g[b,u] v[b,(t+u) % L, c]             (circular correlation)

Key change vs the FFT formulation: mv is computed WITHOUT any FFT.
  M'[t,s] = sum_c k[t,c] q[s,c]  (one 512^3 bf16 matmul per batch — half the
            PE work of the two DFT matmuls, and no DVE spectra products)
  mv[tau] = (1/C) sum_t M'[t, (t+tau) % L]
The diagonal sum is done by accumulating the four 128-row blocks of M' into
one PSUM tile with per-block column rotations (free: column-sliced matmul
outputs), round-tripping the folded P[128,512] through DRAM with a
(row-stride+1) diagonal read, and one ones-column matvec.

delays[0] comes from batch 0 only: instead of a collective (15us fixed cost
in the cost model), every core redundantly computes batch-0's M' block from
a replicated q0/k0 copy (3.4us PE, fully overlapped).

The delay-aggregation circulant is block-circulant with only FOUR distinct
[128,128] stationary blocks, so the stage-C stationary read is [128,512]
(128KB) instead of [128,2048], from a broadcast doubled-g row in DRAM via a
stride-1023 diagonal AP.

Rank codes use a single cnt_gt-256 encoding, exact in bf16 (|code| <= 256),
so the 16 equality masks run in DVE 2x/4x mode and the masks/g matmuls move
bf16. Emission order is the schedule (per-engine queues are in-order): PE
runs warmup -> M-blocks -> rank matvecs -> g matvecs -> stage C with
measured-readiness interleave so it never idles after ramp-up.
"""

import sys
for _p in ('/opt/trn_rl_repo',):
    if _p not in sys.path:
        sys.path.insert(0, _p)

import numpy as np
import ml_dtypes
from contextlib import ExitStack

import concourse.bass as bass
import concourse.bacc as bacc
import concourse.tile as tile
import concourse.mybir as mybir
from concourse.bass_utils import run_bass_kernel_spmd

F32 = mybir.dt.float32
F32R = mybir.dt.float32r
BF16 = mybir.dt.bfloat16
AL = mybir.AluOpType
AF = mybir.ActivationFunctionType
BF = ml_dtypes.bfloat16

B, L, H, E = 32, 512, 8, 64
C = H * E          # 512 channels per batch
NCORES = 8
NB = B // NCORES   # 4 local batches per core

N_WARM = 28        # PE warmup matmuls (p-state ramp while first load lands)


def _consts():
    # packed small consts: [antiI | two | one] -> [128, 130] f32
    small = np.zeros((128, 130), np.float32)
    small[np.arange(128), 127 - np.arange(128)] = 1.0   # anti-identity
    small[:, 128] = 2.0
    small[0, 129] = 1.0
    invc = np.zeros((128, 2), dtype=BF)
    invc[:, 0] = BF(1.0 / C)                            # 2^-9, exact in bf16
    invc[:, 1] = BF(1.0)
    return small, invc


_NC_CACHE = None
PE_LABELS = []
DMA_LABELS = {}


def _label_dmas(nc):
    import traceback
    for eng in (nc.sync, nc.scalar, nc.gpsimd, nc.vector):
        real = eng.dma_start
        def wrapped(*a, _real=real, _eng=eng, **kw):
            inst = _real(*a, **kw)
            frames = traceback.extract_stack()
            lab = "?"
            for fr in reversed(frames):
                if fr.filename.endswith("kernel.py") and fr.name != "wrapped":
                    lab = f"{fr.name}:{fr.lineno}"
                    break
            try:
                DMA_LABELS[inst.name] = f"{_eng.engine.name}:{lab}"
            except Exception:
                pass
            return inst
        eng.dma_start = wrapped


def _label_matmuls(nc):
    real = nc.tensor.matmul
    def wrapped(*a, **kw):
        import traceback
        frames = traceback.extract_stack()
        lab = "?"
        for fr in reversed(frames):
            if fr.filename.endswith("kernel.py") and fr.name != "wrapped":
                lab = f"{fr.name}:{fr.lineno}"
                break
        PE_LABELS.append(lab)
        return real(*a, **kw)
    nc.tensor.matmul = wrapped


def _col0(colap):
    """[128,1] tile-column AP replicated to a [128,128] stationary via a
    stride-0 free dim (PE broadcasts the column to all output partitions)."""
    return bass.AP(tensor=colap.tensor, offset=colap.offset,
                   ap=[[colap.ap[0][0], 128], [0, 128]])


def _build():
    global _NC_CACHE
    if _NC_CACHE is not None:
        return _NC_CACHE
    small_np, invc_np = _consts()

    nc = bacc.Bacc("TRN2", target_bir_lowering=False, debug=False, num_devices=NCORES)
    _label_matmuls(nc)
    tc = tile.TileContext(nc)

    # qk packed [slot, cchunk(4), {q,k}(2), p(128), t(L)] with q/k transposed
    # to [channel, time] on host so channels are the matmul contraction dim.
    qk_all = nc.dram_tensor("qk_all", [NB, 4, 2, 128, L], BF16, kind="ExternalInput")
    qk0_all = nc.dram_tensor("qk0_all", [4, 2, 128, L], BF16, kind="ExternalInput")
    v_all = nc.dram_tensor("v_all", [NB, L, C], BF16, kind="ExternalInput")
    out_all = nc.dram_tensor("out_all", [NB, L, C], BF16, kind="ExternalOutput")

    small_d = nc.inline_tensor(small_np, "small_d")
    invc_d = nc.inline_tensor(invc_np, "invc_d")

    with tc, ExitStack() as ctx:
        cpool = ctx.enter_context(tc.tile_pool(name="consts", bufs=1))
        qpool = ctx.enter_context(tc.tile_pool(name="qk", bufs=1))
        vpool = ctx.enter_context(tc.tile_pool(name="vv", bufs=1))
        wpool = ctx.enter_context(tc.tile_pool(name="work", bufs=1))
        pM = ctx.enter_context(tc.tile_pool(name="pM", bufs=1, space="PSUM"))
        pC = ctx.enter_context(tc.tile_pool(name="pC", bufs=1, space="PSUM"))
        psm = ctx.enter_context(tc.tile_pool(name="psm", bufs=1, space="PSUM"))
        dpool = ctx.enter_context(tc.tile_pool(name="dscratch", bufs=1, space="DRAM"))

        # ---- constants ----
        small_t = cpool.tile([128, 130], F32, name="small_t")
        antiI_t = small_t[:, 0:128]
        one_t = small_t[0:1, 129:130]
        invc_t = cpool.tile([128, 2], BF16, name="invc_t")
        onebf_t = invc_t[0:1, 1:2]

        # ---- PE warmup: ramp the p-state while the first loads land ----
        # (junk tile is memset on DVE at t=0: no DMA dependency; output goes
        #  into the Mps ring, never read, reset by M0's start=True)
        junk_t = cpool.tile([128, 128], BF16, name="junk_t")
        nc.vector.memset(junk_t[:], 0.0)
        warm_ps = pM.tile([128, 512], F32, name="warm_ps", tag="Mps", bufs=2)
        for w in range(N_WARM):
            nc.tensor.matmul(warm_ps[:, 0:128], junk_t[:], junk_t[:],
                             start=True, stop=True)

        # ---- loads ----
        def load_qk_half(dram, base_off, qksb, h):
            nc.sync.dma_start(
                qksb[:, 2048 * h:2048 * (h + 1)],
                bass.AP(tensor=dram, offset=base_off + h * 2 * 2 * 128 * L,
                        ap=[[L, 128], [128 * L, 4], [1, L]]))

        def load_v(s):
            vsb = vpool.tile([128, 2048], BF16, name=f"v_s{s}", tag="vt", bufs=4)
            for qq in range(4):
                nc.sync.dma_start(vsb[:, 512 * qq:512 * (qq + 1)],
                                  bass.AP(tensor=v_all, offset=s * L * C + 128 * qq * C,
                                          ap=[[C, 128], [1, C]]))
            return vsb

        # ---- M' = k q^T with rotated fold into P (one PSUM tile) ----
        # M'[t,s] = sum_c k[t,c] q[s,c];  P[p,u] = sum_i M'[128i+p, (u+128i)%512]
        def emit_M(qksb, nm, ccs=(0, 1, 2, 3), M_ps=None):
            if M_ps is None:
                M_ps = pM.tile([128, 512], F32, name=f"M_{nm}", tag="Mps", bufs=2)
            for cc in ccs:
                qb = 1024 * cc
                kb = 1024 * cc + 512
                for i in range(4):
                    lhs = qksb[:, kb + 128 * i: kb + 128 * i + 128]
                    first = (cc == 0 and i == 0)
                    last = (cc == 3 and i == 3)
                    if i == 0:
                        nc.tensor.matmul(M_ps[:, 0:512], lhs, qksb[:, qb:qb + 512],
                                         start=first, stop=False, skip_group_check=True)
                    else:
                        w = 512 - 128 * i
                        nc.tensor.matmul(M_ps[:, 0:w], lhs, qksb[:, qb + 128 * i:qb + 512],
                                         start=False, stop=last, skip_group_check=True)
                        nc.tensor.matmul(M_ps[:, w:512], lhs, qksb[:, qb:qb + 128 * i],
                                         start=False, stop=last, skip_group_check=True)
            return M_ps

        # ---- P diag round trip: ACT copy + write, Pool read ----
        def emit_Pdiag_w(M_ps, nm):
            with tc.high_priority():
                P_sb = wpool.tile([128, 640], BF16, name=f"P_{nm}", tag="Psb", bufs=2)
                nc.scalar.copy(P_sb[:, 0:512], M_ps[:])
                nc.vector.tensor_copy(P_sb[:, 512:640], M_ps[:, 0:128])
                P_d = dpool.tile([128, 640], BF16, name=f"Pd_{nm}", tag="Pd", bufs=2)
                nc.scalar.dma_start(P_d[:], P_sb[:])
            return P_d[:].tensor

        def emit_Pdiag_r(pd, nm):
            # SAME queue as the write: same DMA ring => ordered without a sem
            with tc.high_priority():
                R_sb = wpool.tile([128, 512], BF16, name=f"R_{nm}", tag="Rsb", bufs=2)
                nc.scalar.dma_start(R_sb[:], bass.AP(tensor=pd, offset=0,
                                                     ap=[[641, 128], [1, 512]]))
            return R_sb

        # ---- mv broadcast to all 128 partitions via stride-0 stationary ----
        def emit_mv(R_sb, nm):
            return _hp(_emit_mv, R_sb, nm)

        def _emit_mv(R_sb, nm):
            mv_ps = psm.tile([128, 512], F32, name=f"mvps_{nm}", tag="psm", bufs=2)
            nc.tensor.matmul(mv_ps[:], _col0(invc_t[:, 0:1]), R_sb[:], start=True, stop=True)
            mvB = wpool.tile([128, 512], F32, name=f"mvB_{nm}", tag="mvB", bufs=2)
            nc.vector.tensor_copy(mvB[:], mv_ps[:])
            return mvB

        def emit_mvT(mvB, nm):
            return _hp(_emit_mvT, mvB, nm)

        def _emit_mvT(mvB, nm):
            mvT_ps = psm.tile([128, 4], F32, name=f"mvTps_{nm}", tag="psT", bufs=1)
            for j in range(4):
                nc.tensor.transpose(mvT_ps[:, j:j + 1], mvB[0:1, 128 * j:128 * (j + 1)], one_t)
            mvT_sb = wpool.tile([128, 4], F32, name=f"mvT_{nm}", tag="mvT", bufs=5)
            nc.vector.tensor_copy(mvT_sb[:], mvT_ps[:])
            return mvT_sb

        # ---- per-slot rank codes (cnt_gt - 256 encoding, all on DVE) ----
        def finish_rank(s, mvB, mvT_sb):
            return _hp(_finish_rank, s, mvB, mvT_sb)

        def _finish_rank(s, mvB, mvT_sb):
            rs = wpool.tile([128, 4], F32, name=f"rs_{s}", tag="rs", bufs=5)
            sgnscr = wpool.tile([128, 512], F32, name=f"sgn_{s}", tag="sgn", bufs=2)
            for j in range(4):
                nc.vector.tensor_scalar(sgnscr[:], mvB[:], mvT_sb[:, j:j + 1], None,
                                        AL.is_gt, AL.add, accum_out=rs[:, j:j + 1])
            # cnt_gt-256: bf16-exact integer in [-256,255]
            rsa = wpool.tile([128, 4], F32, name=f"rsa_{s}", tag="rsa", bufs=5)
            nc.vector.tensor_scalar(rsa[:], rs[:], -256.0, None, AL.add)
            rank_res[s] = rsa
            return rsa

        def finish_soft(s, mvB, mvT_sb):
            expz = wpool.tile([1, 512], F32, name=f"expz_{s}", tag="expz", bufs=2)
            z_sb = wpool.tile([1, 1], F32, name=f"z_{s}", tag="z", bufs=4)
            nc.scalar.activation(expz[:], mvB[0:1, :], AF.Exp, accum_out=z_sb[:])
            z_ps = psm.tile([128, 4], F32, name=f"zps_{s}", tag="psT", bufs=1)
            nc.tensor.matmul(z_ps[:, 0:1],
                             bass.AP(tensor=z_sb[:].tensor, offset=z_sb[:].offset,
                                     ap=[[z_sb[:].ap[0][0], 1], [0, 128]]),
                             one_t, start=True, stop=True)
            rz128 = wpool.tile([128, 1], F32, name=f"rz128_{s}", tag="rz128", bufs=4)
            nc.vector.reciprocal(rz128[:], z_ps[:, 0:1])
            smc = wpool.tile([128, 4], BF16, name=f"smc_{s}", tag="smc", bufs=4)
            nc.scalar.activation(smc[:], mvT_sb[:], AF.Exp)
            soft_res[s] = (rz128, smc)

        def emit_wt(s):
            return _hp(_emit_wt, s)

        def _emit_wt(s):
            rsa = rank_res[s]
            wts = []
            for j in range(4):
                wt = wpool.tile([128, 512], BF16, name=f"wt_{s}_{j}", tag=f"wt{j}", bufs=2)
                eng = nc.vector if j < 2 else nc.gpsimd
                eng.tensor_scalar(wt[:], n2bB[:], rsa[:, j:j + 1], None, AL.is_equal)
                wts.append(wt)
            wt_res[s] = wts

        # ---- g (row-replicated via stride-0 smc columns) -> gmat -> cg4 ----
        def emit_gchain(s):
            rz, smc = soft_res[s]
            wts = wt_res[s]
            g_ps = psm.tile([128, 512], F32, name=f"gps_{s}", tag="psm", bufs=2)
            for j in range(4):
                nc.tensor.matmul(g_ps[:], _col0(smc[:, j:j + 1]), wts[j][:],
                                 start=(j == 0), stop=(j == 3))
            # gnB cols [128:640] = g; [0:128] = g[384:512] tail so the doubled-row
            # window [384,1024) of gmat is one contiguous write
            gnB = wpool.tile([128, 640], BF16, name=f"gnB_{s}", tag="gnB", bufs=2)
            nc.vector.tensor_scalar(gnB[:, 128:640], g_ps[:], rz[:], None, AL.mult)
            nc.vector.tensor_scalar(gnB[:, 0:128], g_ps[:, 384:512], rz[:], None, AL.mult)
            gmat = dpool.tile([128, 1024], BF16, name=f"gmat_{s}", tag="gmat", bufs=2)
            gd = gmat[:].tensor
            nc.scalar.dma_start(bass.AP(tensor=gd, offset=384, ap=[[1024, 128], [1, 640]]),
                                gnB[:])
            cg4 = wpool.tile([128, 512], BF16, name=f"cg4_{s}", tag="cg4", bufs=2)
            nc.scalar.dma_start(cg4[:], bass.AP(tensor=gd, offset=511, ap=[[1023, 128], [1, 512]]))
            chain_res[s] = cg4

        # ---- stage C: block-circulant matmul, 4 distinct stationary blocks ----
        def emit_stagec(s, vsb, split_out=False):
            cg4 = chain_res[s]
            o_sb = wpool.tile([128, 2048], BF16, name=f"osb_{s}", tag="osb", bufs=2)
            for tt in range(4):
                o_ps = pC.tile([128, 512], F32, name=f"ops_{s}_{tt}", tag="ops", bufs=2)
                for ss in range(4):
                    m = (tt - ss) % 4
                    nc.tensor.matmul(o_ps[:], cg4[:, 128 * m:128 * (m + 1)],
                                     vsb[:, 512 * ss:512 * (ss + 1)],
                                     start=(ss == 0), stop=(ss == 3))
                if tt % 2 == 0:
                    nc.scalar.copy(o_sb[:, 512 * tt:512 * (tt + 1)], o_ps[:])
                else:
                    nc.vector.tensor_copy(o_sb[:, 512 * tt:512 * (tt + 1)], o_ps[:])
                if split_out:
                    nc.sync.dma_start(
                        bass.AP(tensor=out_all, offset=s * L * C + 128 * tt * C,
                                ap=[[C, 128], [1, C]]),
                        o_sb[:, 512 * tt:512 * (tt + 1)])
            if not split_out:
                nc.sync.dma_start(
                    bass.AP(tensor=out_all, offset=s * L * C,
                            ap=[[C, 128], [128 * C, 4], [1, C]]),
                    o_sb[:])

        rank_res, soft_res, wt_res, chain_res = {}, {}, {}, {}

        def _hp(fn, *a):
            with tc.high_priority():
                return fn(*a)

        # ================= emission schedule =================
        # Emission order IS the per-engine execution order. Queues: SP = bulk
        # loads (ring bufs=2 throttles them to just-in-time) + out stores;
        # ACT = PSUM->SBUF copies + DRAM writes; Pool = latency-critical DRAM
        # reads (its SEQ waits block nothing else); DVE = the rank/mask/g
        # elementwise chain.
        qk0sb = qpool.tile([128, 4096], BF16, name="qk0sb", tag="qkt", bufs=2)
        qksbs = [qpool.tile([128, 4096], BF16, name=f"qksb_{s}", tag="qkt", bufs=2)
                 for s in range(NB)]

        load_qk_half(qk0_all, 0, qk0sb, 0)
        nc.sync.dma_start(invc_t[:], invc_d.ap())
        load_qk_half(qk0_all, 0, qk0sb, 1)
        nc.sync.dma_start(small_t[:], small_d.ap())
        load_qk_half(qk_all, 0 * 1024 * L, qksbs[0], 0)
        load_qk_half(qk_all, 0 * 1024 * L, qksbs[0], 1)
        load_qk_half(qk_all, 1 * 1024 * L, qksbs[1], 0)
        load_qk_half(qk_all, 1 * 1024 * L, qksbs[1], 1)

        M0 = emit_M(qk0sb, "b0", ccs=(0, 1))
        emit_M(qk0sb, "b0", ccs=(2, 3), M_ps=M0)
        pd0 = emit_Pdiag_w(M0, "b0")
        R0 = emit_Pdiag_r(pd0, "b0")

        Ms = [None] * NB
        R_s = [None] * NB
        Ms[0] = emit_M(qksbs[0], "s0")
        R_s[0] = emit_Pdiag_r(emit_Pdiag_w(Ms[0], "s0"), "s0")
        load_qk_half(qk_all, 2 * 1024 * L, qksbs[2], 0)
        load_qk_half(qk_all, 2 * 1024 * L, qksbs[2], 1)

        Ms[1] = emit_M(qksbs[1], "s1")
        mvB0 = emit_mv(R0, "b0")
        posmvT = emit_mvT(mvB0, "b0")
        R_s[1] = emit_Pdiag_r(emit_Pdiag_w(Ms[1], "s1"), "s1")
        load_qk_half(qk_all, 3 * 1024 * L, qksbs[3], 0)
        load_qk_half(qk_all, 3 * 1024 * L, qksbs[3], 1)

        Ms[2] = emit_M(qksbs[2], "s2", ccs=(0, 1))
        rsa0 = finish_rank("b0", mvB0, posmvT)
        # batch-0 code row for REVERSED positions via anti-identity transposes
        n2bb_ps = psm.tile([1, 512], F32, name="n2bb_ps", bufs=1)
        for j in range(4):
            nc.tensor.matmul(n2bb_ps[0:1, 128 * (3 - j):128 * (4 - j)],
                             rsa0[:, j:j + 1], antiI_t, start=True, stop=True)
        n2bb_row = wpool.tile([1, 512], BF16, name="n2bb_row", bufs=1)
        nc.vector.tensor_copy(n2bb_row[:], n2bb_ps[:])
        n2bB_ps = psm.tile([128, 512], F32, name="n2bB_ps", tag="psm", bufs=2)
        nc.tensor.matmul(n2bB_ps[:], bass.AP(tensor=onebf_t.tensor, offset=onebf_t.offset,
                                             ap=[[onebf_t.ap[0][0], 1], [0, 128]]),
                         n2bb_row[:], start=True, stop=True)
        n2bB = wpool.tile([128, 512], BF16, name="n2bB", bufs=1)
        nc.vector.tensor_copy(n2bB[:], n2bB_ps[:])
        emit_M(qksbs[2], "s2", ccs=(2, 3), M_ps=Ms[2])
        R_s[2] = emit_Pdiag_r(emit_Pdiag_w(Ms[2], "s2"), "s2")

        Ms[3] = emit_M(qksbs[3], "s3", ccs=(0, 1))
        mvB_s0 = emit_mv(R_s[0], "s0")
        mvT_s0 = emit_mvT(mvB_s0, "s0")
        finish_rank(0, mvB_s0, mvT_s0)
        emit_wt(0)
        finish_soft(0, mvB_s0, mvT_s0)
        emit_M(qksbs[3], "s3", ccs=(2, 3), M_ps=Ms[3])
        R_s[3] = emit_Pdiag_r(emit_Pdiag_w(Ms[3], "s3"), "s3")
        vsbs = [load_v(0)]

        emit_gchain(0)
        mvB_s1 = emit_mv(R_s[1], "s1")
        mvT_s1 = emit_mvT(mvB_s1, "s1")
        finish_rank(1, mvB_s1, mvT_s1)
        emit_wt(1)
        finish_soft(1, mvB_s1, mvT_s1)
        vsbs.append(load_v(1))

        mvB_s2 = emit_mv(R_s[2], "s2")
        mvT_s2 = emit_mvT(mvB_s2, "s2")
        emit_gchain(1)
        finish_rank(2, mvB_s2, mvT_s2)
        emit_wt(2)
        finish_soft(2, mvB_s2, mvT_s2)
        vsbs.append(load_v(2))

        emit_stagec(0, vsbs[0])
        mvB_s3 = emit_mv(R_s[3], "s3")
        mvT_s3 = emit_mvT(mvB_s3, "s3")
        emit_gchain(2)
        finish_rank(3, mvB_s3, mvT_s3)
        emit_wt(3)
        finish_soft(3, mvB_s3, mvT_s3)
        vsbs.append(load_v(3))
        emit_stagec(1, vsbs[1])
        emit_gchain(3)
        emit_stagec(2, vsbs[2])
        emit_stagec(3, vsbs[3], split_out=True)

    nc.compile()
    _NC_CACHE = nc
    return nc


def kernel(queries, keys, values):
    q = np.ascontiguousarray(queries, dtype=np.float32).reshape(B, L, C)
    k = np.ascontiguousarray(keys, dtype=np.float32).reshape(B, L, C)
    v = np.ascontiguousarray(values, dtype=np.float32).reshape(B, L, C).astype(BF)
    # [B, C, L] -> [B, cchunk, {q,k}, 128, L]
    qT = np.ascontiguousarray(q.transpose(0, 2, 1)).astype(BF).reshape(B, 4, 128, L)
    kT = np.ascontiguousarray(k.transpose(0, 2, 1)).astype(BF).reshape(B, 4, 128, L)
    qk = np.stack([qT, kT], axis=2)  # [B, 4, 2, 128, L]
    nc = _build()
    in_maps = []
    for c in range(NCORES):
        sl = slice(NB * c, NB * (c + 1))
        in_maps.append({
            "qk_all": qk[sl],
            "qk0_all": qk[0],
            "v_all": v[sl],
        })
    res = run_bass_kernel_spmd(nc, in_maps, core_ids=list(range(NCORES)))
    out = np.concatenate([np.asarray(res.results[c]["out_all"]).astype(np.float32)
                          for c in range(NCORES)], axis=0)
    return out.reshape(B, L, H, E)


if __name__ == "__main__":
    rng = np.random.default_rng(0)
    qq = rng.standard_normal((B, L, H, E)).astype(np.float32)
    kk = rng.standard_normal((B, L, H, E)).astype(np.float32)
    vv = rng.standard_normal((B, L, H, E)).astype(np.float32)
    o = kernel(queries=qq, keys=kk, values=vv)
    print(o.shape, o.dtype, np.abs(o).max())


# revision 33
# speedup vs baseline: 1.2403x; 1.0404x over previous
"""AutoCorrelation (channel-mean circular cross-correlation + rank-matched
delay aggregation) on 8 NeuronCores, pure data parallel over batch.

Math (per batch b, channels c = (h,e), C = 512, L = 512):
  mv[tau]  = (1/C) sum_c sum_t q_c[t] k_c[(t-tau) % L]    (= mean irfft(Q conj K))
  rank0    = descending ranks of mv[batch 0]
  g[b, j]  = softmax(mv[b])_sorted[ rank0[j] ]            (rank-matched scatter)
  out[b,t,c] = sum_u g[b,u] v[b,(t+u) % L, c]             (circular correlation)

Key change vs the FFT formulation: mv is computed WITHOUT any FFT.
  M'[t,s] = sum_c k[t,c] q[s,c]  (one 512^3 bf16 matmul per batch — half the
            PE work of the two DFT matmuls, and no DVE spectra products)
  mv[tau] = (1/C) sum_t M'[t, (t+tau) % L]
The diagonal sum is done by accumulating the four 128-row blocks of M' into
one PSUM tile with per-block column rotations (free: column-sliced matmul
outputs), round-tripping the folded P[128,512] through DRAM with a
(row-stride+1) diagonal read, and one ones-column matvec.

delays[0] comes from batch 0 only: instead of a collective (15us fixed cost
in the cost model), every core redundantly computes batch-0's M' block from
a replicated q0/k0 copy (3.4us PE, fully overlapped).

The delay-aggregation circulant is block-circulant with only FOUR distinct
[128,128] stationary blocks, so the stage-C stationary read is [128,512]
(128KB) instead of [128,2048], from a broadcast doubled-g row in DRAM via a
stride-1023 diagonal AP.

Rank codes use a single cnt_gt-256 encoding, exact in bf16 (|code| <= 256),
so the 16 equality masks run in DVE 2x/4x mode and the masks/g matmuls move
bf16. Emission order is the schedule (per-engine queues are in-order): PE
runs warmup -> M-blocks -> rank matvecs -> g matvecs -> stage C with
measured-readiness interleave so it never idles after ramp-up.
"""

import sys
for _p in ('/opt/trn_rl_repo',):
    if _p not in sys.path:
        sys.path.insert(0, _p)

import numpy as np
import ml_dtypes
from contextlib import ExitStack

import concourse.bass as bass
import concourse.bacc as bacc
import concourse.tile as tile
import concourse.mybir as mybir
from concourse.bass_utils import run_bass_kernel_spmd

F32 = mybir.dt.float32
F32R = mybir.dt.float32r
BF16 = mybir.dt.bfloat16
AL = mybir.AluOpType
AF = mybir.ActivationFunctionType
BF = ml_dtypes.bfloat16

B, L, H, E = 32, 512, 8, 64
C = H * E          # 512 channels per batch
NCORES = 8
NB = B // NCORES   # 4 local batches per core

N_WARM = 28        # PE warmup matmuls (p-state ramp while first load lands)


def _consts():
    # packed small consts: [antiI | two | one] -> [128, 130] f32
    small = np.zeros((128, 130), np.float32)
    small[np.arange(128), 127 - np.arange(128)] = 1.0   # anti-identity
    small[:, 128] = 2.0
    small[0, 129] = 1.0
    invc = np.zeros((128, 2), dtype=BF)
    invc[:, 0] = BF(1.0 / C)                            # 2^-9, exact in bf16
    invc[:, 1] = BF(1.0)
    return small, invc


_NC_CACHE = None
PE_LABELS = []
DMA_LABELS = {}


def _label_dmas(nc):
    import traceback
    for eng in (nc.sync, nc.scalar, nc.gpsimd, nc.vector):
        real = eng.dma_start
        def wrapped(*a, _real=real, _eng=eng, **kw):
            inst = _real(*a, **kw)
            frames = traceback.extract_stack()
            lab = "?"
            for fr in reversed(frames):
                if fr.filename.endswith("kernel.py") and fr.name != "wrapped":
                    lab = f"{fr.name}:{fr.lineno}"
                    break
            try:
                DMA_LABELS[inst.name] = f"{_eng.engine.name}:{lab}"
            except Exception:
                pass
            return inst
        eng.dma_start = wrapped


def _label_matmuls(nc):
    real = nc.tensor.matmul
    def wrapped(*a, **kw):
        import traceback
        frames = traceback.extract_stack()
        lab = "?"
        for fr in reversed(frames):
            if fr.filename.endswith("kernel.py") and fr.name != "wrapped":
                lab = f"{fr.name}:{fr.lineno}"
                break
        PE_LABELS.append(lab)
        return real(*a, **kw)
    nc.tensor.matmul = wrapped


def _col0(colap):
    """[128,1] tile-column AP replicated to a [128,128] stationary via a
    stride-0 free dim (PE broadcasts the column to all output partitions)."""
    return bass.AP(tensor=colap.tensor, offset=colap.offset,
                   ap=[[colap.ap[0][0], 128], [0, 128]])


def _build():
    global _NC_CACHE
    if _NC_CACHE is not None:
        return _NC_CACHE
    small_np, invc_np = _consts()

    nc = bacc.Bacc("TRN2", target_bir_lowering=False, debug=False, num_devices=NCORES)
    _label_matmuls(nc)
    tc = tile.TileContext(nc)

    # qk packed [slot, cchunk(4), {q,k}(2), p(128), t(L)] with q/k transposed
    # to [channel, time] on host so channels are the matmul contraction dim.
    qk_all = nc.dram_tensor("qk_all", [NB, 4, 2, 128, L], BF16, kind="ExternalInput")
    qk0_all = nc.dram_tensor("qk0_all", [4, 2, 128, L], BF16, kind="ExternalInput")
    v_all = nc.dram_tensor("v_all", [NB, L, C], BF16, kind="ExternalInput")
    out_all = nc.dram_tensor("out_all", [NB, L, C], BF16, kind="ExternalOutput")

    small_d = nc.inline_tensor(small_np, "small_d")
    invc_d = nc.inline_tensor(invc_np, "invc_d")

    with tc, ExitStack() as ctx:
        cpool = ctx.enter_context(tc.tile_pool(name="consts", bufs=1))
        qpool = ctx.enter_context(tc.tile_pool(name="qk", bufs=1))
        vpool = ctx.enter_context(tc.tile_pool(name="vv", bufs=1))
        wpool = ctx.enter_context(tc.tile_pool(name="work", bufs=1))
        pM = ctx.enter_context(tc.tile_pool(name="pM", bufs=1, space="PSUM"))
        pC = ctx.enter_context(tc.tile_pool(name="pC", bufs=1, space="PSUM"))
        psm = ctx.enter_context(tc.tile_pool(name="psm", bufs=1, space="PSUM"))
        dpool = ctx.enter_context(tc.tile_pool(name="dscratch", bufs=1, space="DRAM"))

        # ---- constants ----
        small_t = cpool.tile([128, 130], F32, name="small_t")
        antiI_t = small_t[:, 0:128]
        one_t = small_t[0:1, 129:130]
        invc_t = cpool.tile([128, 2], BF16, name="invc_t")
        onebf_t = invc_t[0:1, 1:2]

        # ---- PE warmup: ramp the p-state while the first loads land ----
        # (junk tile is memset on DVE at t=0: no DMA dependency; output goes
        #  into the Mps ring, never read, reset by M0's start=True)
        junk_t = cpool.tile([128, 128], BF16, name="junk_t")
        nc.vector.memset(junk_t[:], 0.0)
        warm_ps = pM.tile([128, 512], F32, name="warm_ps", tag="Mps", bufs=2)
        for w in range(N_WARM):
            nc.tensor.matmul(warm_ps[:, 0:128], junk_t[:], junk_t[:],
                             start=True, stop=True)

        # ---- loads ----
        def load_qk_half(dram, base_off, qksb, h):
            nc.sync.dma_start(
                qksb[:, 2048 * h:2048 * (h + 1)],
                bass.AP(tensor=dram, offset=base_off + h * 2 * 2 * 128 * L,
                        ap=[[L, 128], [128 * L, 4], [1, L]]))

        def load_v(s):
            vsb = vpool.tile([128, 2048], BF16, name=f"v_s{s}", tag="vt", bufs=4)
            for qq in range(4):
                nc.sync.dma_start(vsb[:, 512 * qq:512 * (qq + 1)],
                                  bass.AP(tensor=v_all, offset=s * L * C + 128 * qq * C,
                                          ap=[[C, 128], [1, C]]))
            return vsb

        # ---- M' = k q^T with rotated fold into P (one PSUM tile) ----
        # M'[t,s] = sum_c k[t,c] q[s,c];  P[p,u] = sum_i M'[128i+p, (u+128i)%512]
        def emit_M(qksb, nm, ccs=(0, 1, 2, 3), M_ps=None):
            if M_ps is None:
                M_ps = pM.tile([128, 512], F32, name=f"M_{nm}", tag="Mps", bufs=2)
            for cc in ccs:
                qb = 1024 * cc
                kb = 1024 * cc + 512
                for i in range(4):
                    lhs = qksb[:, kb + 128 * i: kb + 128 * i + 128]
                    first = (cc == 0 and i == 0)
                    last = (cc == 3 and i == 3)
                    if i == 0:
                        nc.tensor.matmul(M_ps[:, 0:512], lhs, qksb[:, qb:qb + 512],
                                         start=first, stop=False, skip_group_check=True)
                    else:
                        w = 512 - 128 * i
                        nc.tensor.matmul(M_ps[:, 0:w], lhs, qksb[:, qb + 128 * i:qb + 512],
                                         start=False, stop=last, skip_group_check=True)
                        nc.tensor.matmul(M_ps[:, w:512], lhs, qksb[:, qb:qb + 128 * i],
                                         start=False, stop=last, skip_group_check=True)
            return M_ps

        # ---- P diag round trip: ACT copy + write, Pool read ----
        def emit_Pdiag_w(M_ps, nm):
            with _hpctx():
                P_sb = wpool.tile([128, 640], BF16, name=f"P_{nm}", tag="Psb", bufs=2)
                nc.scalar.copy(P_sb[:, 0:512], M_ps[:])
                nc.vector.tensor_copy(P_sb[:, 512:640], M_ps[:, 0:128])
                P_d = dpool.tile([128, 640], BF16, name=f"Pd_{nm}", tag="Pd", bufs=2)
                nc.scalar.dma_start(P_d[:], P_sb[:])
            return P_d[:].tensor

        def emit_Pdiag_r(pd, nm):
            # SAME queue as the write: same DMA ring => ordered without a sem
            with _hpctx():
                R_sb = wpool.tile([128, 512], BF16, name=f"R_{nm}", tag="Rsb", bufs=2)
                nc.gpsimd.dma_start(R_sb[:], bass.AP(tensor=pd, offset=0,
                                                     ap=[[641, 128], [1, 512]]))
            return R_sb

        # ---- mv broadcast to all 128 partitions via stride-0 stationary ----
        def emit_mv(R_sb, nm):
            with _hpctx():
                mv_ps = psm.tile([128, 512], F32, name=f"mvps_{nm}", tag="psm", bufs=2)
                nc.tensor.matmul(mv_ps[:], _col0(invc_t[:, 0:1]), R_sb[:], start=True, stop=True)
                mvB = wpool.tile([128, 512], F32, name=f"mvB_{nm}", tag="mvB", bufs=2)
                nc.vector.tensor_copy(mvB[:], mv_ps[:])
            return mvB

        def emit_mvT(mvB, nm):
            with _hpctx():
                mvT_ps = psm.tile([128, 4], F32, name=f"mvTps_{nm}", tag="psT", bufs=1)
                for j in range(4):
                    nc.tensor.transpose(mvT_ps[:, j:j + 1], mvB[0:1, 128 * j:128 * (j + 1)], one_t)
                mvT_sb = wpool.tile([128, 4], F32, name=f"mvT_{nm}", tag="mvT", bufs=5)
                nc.vector.tensor_copy(mvT_sb[:], mvT_ps[:])
            return mvT_sb

        # ---- per-slot rank codes (cnt_gt - 256 encoding, all on DVE) ----
        def finish_rank(s, mvB, mvT_sb):
            return _hp(_finish_rank, s, mvB, mvT_sb)

        def _finish_rank(s, mvB, mvT_sb):
            rs = wpool.tile([128, 4], F32, name=f"rs_{s}", tag="rs", bufs=5)
            sgnscr = wpool.tile([128, 512], F32, name=f"sgn_{s}", tag="sgn", bufs=2)
            for j in range(4):
                nc.vector.tensor_scalar(sgnscr[:], mvB[:], mvT_sb[:, j:j + 1], None,
                                        AL.is_gt, AL.add, accum_out=rs[:, j:j + 1])
            # cnt_gt-256: bf16-exact integer in [-256,255]
            rsa = wpool.tile([128, 4], F32, name=f"rsa_{s}", tag="rsa", bufs=5)
            nc.vector.tensor_scalar(rsa[:], rs[:], -256.0, None, AL.add)
            rank_res[s] = rsa
            return rsa

        def finish_soft(s, mvB, mvT_sb):
            with _hpctx():
                expz = wpool.tile([1, 512], F32, name=f"expz_{s}", tag="expz", bufs=2)
                z_sb = wpool.tile([1, 1], F32, name=f"z_{s}", tag="z", bufs=4)
                nc.scalar.activation(expz[:], mvB[0:1, :], AF.Exp, accum_out=z_sb[:])
                z_ps = psm.tile([128, 4], F32, name=f"zps_{s}", tag="psT", bufs=1)
                nc.tensor.matmul(z_ps[:, 0:1],
                                 bass.AP(tensor=z_sb[:].tensor, offset=z_sb[:].offset,
                                         ap=[[z_sb[:].ap[0][0], 1], [0, 128]]),
                                 one_t, start=True, stop=True)
                rz128 = wpool.tile([128, 1], F32, name=f"rz128_{s}", tag="rz128", bufs=4)
                nc.vector.reciprocal(rz128[:], z_ps[:, 0:1])
                smc = wpool.tile([128, 4], BF16, name=f"smc_{s}", tag="smc", bufs=4)
                nc.scalar.activation(smc[:], mvT_sb[:], AF.Exp)
            soft_res[s] = (rz128, smc)

        def emit_wt(s):
            return _hp(_emit_wt, s)

        def _emit_wt(s):
            rsa = rank_res[s]
            wts = []
            for j in range(4):
                wt = wpool.tile([128, 512], BF16, name=f"wt_{s}_{j}", tag=f"wt{j}", bufs=2)
                nc.vector.tensor_scalar(wt[:], n2bB[:], rsa[:, j:j + 1], None, AL.is_equal)
                wts.append(wt)
            wt_res[s] = wts

        # ---- g (row-replicated via stride-0 smc columns) -> gmat -> cg4 ----
        def emit_gchain(s):
            rz, smc = soft_res[s]
            wts = wt_res[s]
            g_ps = psm.tile([128, 512], F32, name=f"gps_{s}", tag="psm", bufs=2)
            for j in range(4):
                nc.tensor.matmul(g_ps[:], _col0(smc[:, j:j + 1]), wts[j][:],
                                 start=(j == 0), stop=(j == 3))
            # gnB cols [128:640] = g; [0:128] = g[384:512] tail so the doubled-row
            # window [384,1024) of gmat is one contiguous write
            gnB = wpool.tile([128, 640], BF16, name=f"gnB_{s}", tag="gnB", bufs=2)
            nc.vector.tensor_scalar(gnB[:, 128:640], g_ps[:], rz[:], None, AL.mult)
            nc.vector.tensor_scalar(gnB[:, 0:128], g_ps[:, 384:512], rz[:], None, AL.mult)
            gmat = dpool.tile([128, 1024], BF16, name=f"gmat_{s}", tag="gmat", bufs=2)
            gd = gmat[:].tensor
            nc.scalar.dma_start(bass.AP(tensor=gd, offset=384, ap=[[1024, 128], [1, 640]]),
                                gnB[:])
            cg4 = wpool.tile([128, 512], BF16, name=f"cg4_{s}", tag="cg4", bufs=2)
            nc.gpsimd.dma_start(cg4[:], bass.AP(tensor=gd, offset=511, ap=[[1023, 128], [1, 512]]))
            chain_res[s] = cg4

        # ---- stage C: block-circulant matmul, 4 distinct stationary blocks ----
        def emit_stagec(s, vsb, split_out=False):
            cg4 = chain_res[s]
            o_sb = wpool.tile([128, 2048], BF16, name=f"osb_{s}", tag="osb", bufs=2)
            for tt in range(4):
                o_ps = pC.tile([128, 512], F32, name=f"ops_{s}_{tt}", tag="ops", bufs=2)
                for ss in range(4):
                    m = (tt - ss) % 4
                    nc.tensor.matmul(o_ps[:], cg4[:, 128 * m:128 * (m + 1)],
                                     vsb[:, 512 * ss:512 * (ss + 1)],
                                     start=(ss == 0), stop=(ss == 3))
                if tt % 2 == 0:
                    nc.scalar.copy(o_sb[:, 512 * tt:512 * (tt + 1)], o_ps[:])
                else:
                    nc.vector.tensor_copy(o_sb[:, 512 * tt:512 * (tt + 1)], o_ps[:])
                if split_out:
                    nc.sync.dma_start(
                        bass.AP(tensor=out_all, offset=s * L * C + 128 * tt * C,
                                ap=[[C, 128], [1, C]]),
                        o_sb[:, 512 * tt:512 * (tt + 1)])
            if not split_out:
                nc.sync.dma_start(
                    bass.AP(tensor=out_all, offset=s * L * C,
                            ap=[[C, 128], [128 * C, 4], [1, C]]),
                    o_sb[:])

        rank_res, soft_res, wt_res, chain_res = {}, {}, {}, {}

        import contextlib

        def _nullhp():
            return contextlib.nullcontext()
        tc_high_priority_real = tc.high_priority
        USE_HP = True

        def _hpctx():
            return tc_high_priority_real() if USE_HP else contextlib.nullcontext()

        def _hp(fn, *a):
            with _hpctx():
                return fn(*a)

        # ================= emission schedule =================
        # Emission order IS the per-engine execution order. Queues: SP = bulk
        # loads (ring bufs=2 throttles them to just-in-time) + out stores;
        # ACT = PSUM->SBUF copies + DRAM writes; Pool = latency-critical DRAM
        # reads (its SEQ waits block nothing else); DVE = the rank/mask/g
        # elementwise chain.
        qk0sb = qpool.tile([128, 4096], BF16, name="qk0sb", tag="qkt", bufs=2)
        qksbs = [qpool.tile([128, 4096], BF16, name=f"qksb_{s}", tag="qkt", bufs=2)
                 for s in range(NB)]

        load_qk_half(qk0_all, 0, qk0sb, 0)
        nc.sync.dma_start(invc_t[:], invc_d.ap())
        load_qk_half(qk0_all, 0, qk0sb, 1)
        nc.sync.dma_start(small_t[:], small_d.ap())
        load_qk_half(qk_all, 0 * 1024 * L, qksbs[0], 0)
        load_qk_half(qk_all, 0 * 1024 * L, qksbs[0], 1)
        load_qk_half(qk_all, 1 * 1024 * L, qksbs[1], 0)
        load_qk_half(qk_all, 1 * 1024 * L, qksbs[1], 1)

        M0 = emit_M(qk0sb, "b0", ccs=(0, 1))
        emit_M(qk0sb, "b0", ccs=(2, 3), M_ps=M0)
        pd0 = emit_Pdiag_w(M0, "b0")
        R0 = emit_Pdiag_r(pd0, "b0")

        Ms = [None] * NB
        R_s = [None] * NB
        Ms[0] = emit_M(qksbs[0], "s0")
        R_s[0] = emit_Pdiag_r(emit_Pdiag_w(Ms[0], "s0"), "s0")
        load_qk_half(qk_all, 2 * 1024 * L, qksbs[2], 0)
        load_qk_half(qk_all, 2 * 1024 * L, qksbs[2], 1)

        Ms[1] = emit_M(qksbs[1], "s1")
        mvB0 = emit_mv(R0, "b0")
        posmvT = emit_mvT(mvB0, "b0")
        R_s[1] = emit_Pdiag_r(emit_Pdiag_w(Ms[1], "s1"), "s1")
        load_qk_half(qk_all, 3 * 1024 * L, qksbs[3], 0)
        load_qk_half(qk_all, 3 * 1024 * L, qksbs[3], 1)

        Ms[2] = emit_M(qksbs[2], "s2", ccs=(0, 1))
        rsa0 = finish_rank("b0", mvB0, posmvT)
        # batch-0 code row for REVERSED positions via anti-identity transposes
        n2bb_ps = psm.tile([1, 512], F32, name="n2bb_ps", bufs=1)
        for j in range(4):
            nc.tensor.matmul(n2bb_ps[0:1, 128 * (3 - j):128 * (4 - j)],
                             rsa0[:, j:j + 1], antiI_t, start=True, stop=True)
        n2bb_row = wpool.tile([1, 512], BF16, name="n2bb_row", bufs=1)
        nc.vector.tensor_copy(n2bb_row[:], n2bb_ps[:])
        n2bB_ps = psm.tile([128, 512], F32, name="n2bB_ps", tag="psm", bufs=2)
        nc.tensor.matmul(n2bB_ps[:], bass.AP(tensor=onebf_t.tensor, offset=onebf_t.offset,
                                             ap=[[onebf_t.ap[0][0], 1], [0, 128]]),
                         n2bb_row[:], start=True, stop=True)
        n2bB = wpool.tile([128, 512], BF16, name="n2bB", bufs=1)
        nc.vector.tensor_copy(n2bB[:], n2bB_ps[:])
        emit_M(qksbs[2], "s2", ccs=(2, 3), M_ps=Ms[2])
        R_s[2] = emit_Pdiag_r(emit_Pdiag_w(Ms[2], "s2"), "s2")

        Ms[3] = emit_M(qksbs[3], "s3", ccs=(0, 1))
        mvB_s0 = emit_mv(R_s[0], "s0")
        mvT_s0 = emit_mvT(mvB_s0, "s0")
        finish_rank(0, mvB_s0, mvT_s0)
        emit_wt(0)
        finish_soft(0, mvB_s0, mvT_s0)
        emit_M(qksbs[3], "s3", ccs=(2, 3), M_ps=Ms[3])
        R_s[3] = emit_Pdiag_r(emit_Pdiag_w(Ms[3], "s3"), "s3")
        vsbs = [load_v(0)]

        emit_gchain(0)
        mvB_s1 = emit_mv(R_s[1], "s1")
        mvT_s1 = emit_mvT(mvB_s1, "s1")
        finish_rank(1, mvB_s1, mvT_s1)
        emit_wt(1)
        finish_soft(1, mvB_s1, mvT_s1)
        vsbs.append(load_v(1))

        mvB_s2 = emit_mv(R_s[2], "s2")
        mvT_s2 = emit_mvT(mvB_s2, "s2")
        emit_gchain(1)
        finish_rank(2, mvB_s2, mvT_s2)
        emit_wt(2)
        finish_soft(2, mvB_s2, mvT_s2)
        vsbs.append(load_v(2))

        emit_stagec(0, vsbs[0])
        mvB_s3 = emit_mv(R_s[3], "s3")
        mvT_s3 = emit_mvT(mvB_s3, "s3")
        emit_gchain(2)
        finish_rank(3, mvB_s3, mvT_s3)
        emit_wt(3)
        finish_soft(3, mvB_s3, mvT_s3)
        vsbs.append(load_v(3))
        emit_stagec(1, vsbs[1])
        emit_gchain(3)
        emit_stagec(2, vsbs[2])
        emit_stagec(3, vsbs[3], split_out=True)

    nc.compile()
    _NC_CACHE = nc
    return nc


def kernel(queries, keys, values):
    q = np.ascontiguousarray(queries, dtype=np.float32).reshape(B, L, C)
    k = np.ascontiguousarray(keys, dtype=np.float32).reshape(B, L, C)
    v = np.ascontiguousarray(values, dtype=np.float32).reshape(B, L, C).astype(BF)
    # [B, C, L] -> [B, cchunk, {q,k}, 128, L]
    qT = np.ascontiguousarray(q.transpose(0, 2, 1)).astype(BF).reshape(B, 4, 128, L)
    kT = np.ascontiguousarray(k.transpose(0, 2, 1)).astype(BF).reshape(B, 4, 128, L)
    qk = np.stack([qT, kT], axis=2)  # [B, 4, 2, 128, L]
    nc = _build()
    in_maps = []
    for c in range(NCORES):
        sl = slice(NB * c, NB * (c + 1))
        in_maps.append({
            "qk_all": qk[sl],
            "qk0_all": qk[0],
            "v_all": v[sl],
        })
    res = run_bass_kernel_spmd(nc, in_maps, core_ids=list(range(NCORES)))
    out = np.concatenate([np.asarray(res.results[c]["out_all"]).astype(np.float32)
                          for c in range(NCORES)], axis=0)
    return out.reshape(B, L, H, E)


if __name__ == "__main__":
    rng = np.random.default_rng(0)
    qq = rng.standard_normal((B, L, H, E)).astype(np.float32)
    kk = rng.standard_normal((B, L, H, E)).astype(np.float32)
    vv = rng.standard_normal((B, L, H, E)).astype(np.float32)
    o = kernel(queries=qq, keys=kk, values=vv)
    print(o.shape, o.dtype, np.abs(o).max())


# revision 43
# speedup vs baseline: 1.2758x; 1.0286x over previous
"""AutoCorrelation (channel-mean circular cross-correlation + rank-matched
delay aggregation) on 8 NeuronCores, pure data parallel over batch.

Math (per batch b, channels c = (h,e), C = 512, L = 512):
  mv[tau]  = (1/C) sum_c sum_t q_c[t] k_c[(t-tau) % L]    (= mean irfft(Q conj K))
  rank0    = descending ranks of mv[batch 0]
  g[b, j]  = softmax(mv[b])_sorted[ rank0[j] ]            (rank-matched scatter)
  out[b,t,c] = sum_u g[b,u] v[b,(t+u) % L, c]             (circular correlation)

Key change vs the FFT formulation: mv is computed WITHOUT any FFT.
  M'[t,s] = sum_c k[t,c] q[s,c]  (one 512^3 bf16 matmul per batch — half the
            PE work of the two DFT matmuls, and no DVE spectra products)
  mv[tau] = (1/C) sum_t M'[t, (t+tau) % L]
The diagonal sum is done by accumulating the four 128-row blocks of M' into
one PSUM tile with per-block column rotations (free: column-sliced matmul
outputs), round-tripping the folded P[128,512] through DRAM with a
(row-stride+1) diagonal read, and one ones-column matvec.

delays[0] comes from batch 0 only: instead of a collective (15us fixed cost
in the cost model), every core redundantly computes batch-0's M' block from
a replicated q0/k0 copy (3.4us PE, fully overlapped).

The delay-aggregation circulant is block-circulant with only FOUR distinct
[128,128] stationary blocks, so the stage-C stationary read is [128,512]
(128KB) instead of [128,2048], from a broadcast doubled-g row in DRAM via a
stride-1023 diagonal AP.

Rank codes use a single cnt_gt-256 encoding, exact in bf16 (|code| <= 256),
so the 16 equality masks run in DVE 2x/4x mode and the masks/g matmuls move
bf16. Emission order is the schedule (per-engine queues are in-order): PE
runs warmup -> M-blocks -> rank matvecs -> g matvecs -> stage C with
measured-readiness interleave so it never idles after ramp-up.
"""

import sys
for _p in ('/opt/trn_rl_repo',):
    if _p not in sys.path:
        sys.path.insert(0, _p)

import numpy as np
import ml_dtypes
from contextlib import ExitStack

import concourse.bass as bass
import concourse.bacc as bacc
import concourse.tile as tile
import concourse.mybir as mybir
from concourse.bass_utils import run_bass_kernel_spmd

F32 = mybir.dt.float32
F32R = mybir.dt.float32r
BF16 = mybir.dt.bfloat16
AL = mybir.AluOpType
AF = mybir.ActivationFunctionType
BF = ml_dtypes.bfloat16

B, L, H, E = 32, 512, 8, 64
C = H * E          # 512 channels per batch
NCORES = 8
NB = B // NCORES   # 4 local batches per core

N_WARM = 20        # PE warmup matmuls (p-state ramp while first load lands)


def _consts():
    # packed small consts: [antiI | two | one] -> [128, 130] f32
    small = np.zeros((128, 130), np.float32)
    small[np.arange(128), 127 - np.arange(128)] = 1.0   # anti-identity
    small[:, 128] = 2.0
    small[0, 129] = 1.0
    invc = np.zeros((128, 2), dtype=BF)
    invc[:, 0] = BF(1.0 / C)                            # 2^-9, exact in bf16
    invc[:, 1] = BF(1.0)
    return small, invc


_NC_CACHE = None
PE_LABELS = []
DMA_LABELS = {}


def _label_dmas(nc):
    import traceback
    for eng in (nc.sync, nc.scalar, nc.gpsimd, nc.vector):
        real = eng.dma_start
        def wrapped(*a, _real=real, _eng=eng, **kw):
            inst = _real(*a, **kw)
            frames = traceback.extract_stack()
            lab = "?"
            for fr in reversed(frames):
                if fr.filename.endswith("kernel.py") and fr.name != "wrapped":
                    lab = f"{fr.name}:{fr.lineno}"
                    break
            try:
                DMA_LABELS[inst.name] = f"{_eng.engine.name}:{lab}"
            except Exception:
                pass
            return inst
        eng.dma_start = wrapped


def _label_matmuls(nc):
    real = nc.tensor.matmul
    def wrapped(*a, **kw):
        import traceback
        frames = traceback.extract_stack()
        lab = "?"
        for fr in reversed(frames):
            if fr.filename.endswith("kernel.py") and fr.name != "wrapped":
                lab = f"{fr.name}:{fr.lineno}"
                break
        PE_LABELS.append(lab)
        return real(*a, **kw)
    nc.tensor.matmul = wrapped


def _col0(colap):
    """[128,1] tile-column AP replicated to a [128,128] stationary via a
    stride-0 free dim (PE broadcasts the column to all output partitions)."""
    return bass.AP(tensor=colap.tensor, offset=colap.offset,
                   ap=[[colap.ap[0][0], 128], [0, 128]])


def _build():
    global _NC_CACHE
    if _NC_CACHE is not None:
        return _NC_CACHE
    small_np, invc_np = _consts()

    nc = bacc.Bacc("TRN2", target_bir_lowering=False, debug=False, num_devices=NCORES)
    _label_matmuls(nc)
    tc = tile.TileContext(nc)

    # qk packed [slot, cchunk(4), {q,k}(2), p(128), t(L)] with q/k transposed
    # to [channel, time] on host so channels are the matmul contraction dim.
    qk_all = nc.dram_tensor("qk_all", [NB, 4, 2, 128, L], BF16, kind="ExternalInput")
    qk0_all = nc.dram_tensor("qk0_all", [4, 2, 128, L], BF16, kind="ExternalInput")
    v_all = nc.dram_tensor("v_all", [NB, L, C], BF16, kind="ExternalInput")
    out_all = nc.dram_tensor("out_all", [NB, L, C], BF16, kind="ExternalOutput")

    small_d = nc.inline_tensor(small_np, "small_d")
    invc_d = nc.inline_tensor(invc_np, "invc_d")

    with tc, ExitStack() as ctx:
        cpool = ctx.enter_context(tc.tile_pool(name="consts", bufs=1))
        qpool = ctx.enter_context(tc.tile_pool(name="qk", bufs=1))
        vpool = ctx.enter_context(tc.tile_pool(name="vv", bufs=1))
        wpool = ctx.enter_context(tc.tile_pool(name="work", bufs=1))
        pM = ctx.enter_context(tc.tile_pool(name="pM", bufs=1, space="PSUM"))
        pC = ctx.enter_context(tc.tile_pool(name="pC", bufs=1, space="PSUM"))
        psm = ctx.enter_context(tc.tile_pool(name="psm", bufs=1, space="PSUM"))
        dpool = ctx.enter_context(tc.tile_pool(name="dscratch", bufs=1, space="DRAM"))

        # ---- constants ----
        small_t = cpool.tile([128, 130], F32, name="small_t")
        antiI_t = small_t[:, 0:128]
        one_t = small_t[0:1, 129:130]
        invc_t = cpool.tile([128, 2], BF16, name="invc_t")
        onebf_t = invc_t[0:1, 1:2]

        # ---- PE warmup: ramp the p-state while the first loads land ----
        # (junk tile is memset on DVE at t=0: no DMA dependency; output goes
        #  into the Mps ring, never read, reset by M0's start=True)
        junk_t = cpool.tile([128, 128], BF16, name="junk_t")
        nc.vector.memset(junk_t[:], 0.0)
        warm_ps = pM.tile([128, 512], F32, name="warm_ps", tag="Mps", bufs=2)
        for w in range(N_WARM):
            nc.tensor.matmul(warm_ps[:, 0:128], junk_t[:], junk_t[:],
                             start=True, stop=True)

        # ---- loads ----
        def load_qk_half(dram, base_off, qksb, h):
            nc.sync.dma_start(
                qksb[:, 2048 * h:2048 * (h + 1)],
                bass.AP(tensor=dram, offset=base_off + h * 2 * 2 * 128 * L,
                        ap=[[L, 128], [128 * L, 4], [1, L]]))

        def load_v(s):
            vsb = vpool.tile([128, 2048], BF16, name=f"v_s{s}", tag="vt", bufs=4)
            for qq in range(4):
                nc.sync.dma_start(vsb[:, 512 * qq:512 * (qq + 1)],
                                  bass.AP(tensor=v_all, offset=s * L * C + 128 * qq * C,
                                          ap=[[C, 128], [1, C]]))
            return vsb

        # ---- M' = k q^T with rotated fold into P (one PSUM tile) ----
        # M'[t,s] = sum_c k[t,c] q[s,c];  P[p,u] = sum_i M'[128i+p, (u+128i)%512]
        def emit_M(qksb, nm, ccs=(0, 1, 2, 3), M_ps=None):
            if M_ps is None:
                M_ps = pM.tile([128, 512], F32, name=f"M_{nm}", tag="Mps", bufs=2)
            for cc in ccs:
                qb = 1024 * cc
                kb = 1024 * cc + 512
                for i in range(4):
                    lhs = qksb[:, kb + 128 * i: kb + 128 * i + 128]
                    first = (cc == 0 and i == 0)
                    last = (cc == 3 and i == 3)
                    if i == 0:
                        nc.tensor.matmul(M_ps[:, 0:512], lhs, qksb[:, qb:qb + 512],
                                         start=first, stop=False, skip_group_check=True)
                    else:
                        w = 512 - 128 * i
                        nc.tensor.matmul(M_ps[:, 0:w], lhs, qksb[:, qb + 128 * i:qb + 512],
                                         start=False, stop=last, skip_group_check=True)
                        nc.tensor.matmul(M_ps[:, w:512], lhs, qksb[:, qb:qb + 128 * i],
                                         start=False, stop=last, skip_group_check=True)
            return M_ps

        # ---- P diag round trip: ACT copy + write, Pool read ----
        def emit_Pdiag_w(M_ps, nm):
            with _hpctx():
                P_sb = wpool.tile([128, 640], BF16, name=f"P_{nm}", tag="Psb", bufs=2)
                nc.scalar.copy(P_sb[:, 0:512], M_ps[:])
                nc.vector.tensor_copy(P_sb[:, 512:640], M_ps[:, 0:128])
                P_d = dpool.tile([128, 640], BF16, name=f"Pd_{nm}", tag="Pd", bufs=2)
                nc.scalar.dma_start(P_d[:], P_sb[:])
            return P_d[:].tensor

        def emit_Pdiag_r(pd, nm):
            # SAME queue as the write: same DMA ring => ordered without a sem
            with _hpctx():
                R_sb = wpool.tile([128, 512], BF16, name=f"R_{nm}", tag="Rsb", bufs=2)
                nc.gpsimd.dma_start(R_sb[:], bass.AP(tensor=pd, offset=0,
                                                     ap=[[641, 128], [1, 512]]))
            return R_sb

        # ---- mv broadcast to all 128 partitions via stride-0 stationary ----
        def emit_mv(R_sb, nm):
            with _hpctx():
                mv_ps = psm.tile([128, 512], F32, name=f"mvps_{nm}", tag="psm", bufs=2)
                nc.tensor.matmul(mv_ps[:], _col0(invc_t[:, 0:1]), R_sb[:], start=True, stop=True)
                mvB = wpool.tile([128, 512], F32, name=f"mvB_{nm}", tag="mvB", bufs=2)
                nc.scalar.copy(mvB[:], mv_ps[:])
            return mvB

        def emit_mvT(mvB, nm):
            with _hpctx():
                mvT_ps = psm.tile([128, 4], F32, name=f"mvTps_{nm}", tag="psT", bufs=1)
                for j in range(4):
                    nc.tensor.transpose(mvT_ps[:, j:j + 1], mvB[0:1, 128 * j:128 * (j + 1)], one_t)
                mvT_sb = wpool.tile([128, 4], F32, name=f"mvT_{nm}", tag="mvT", bufs=5)
                nc.vector.tensor_copy(mvT_sb[:], mvT_ps[:])
            return mvT_sb

        # ---- per-slot rank codes (cnt_gt - 256 encoding, all on DVE) ----
        def finish_rank(s, mvB, mvT_sb):
            return _hp(_finish_rank, s, mvB, mvT_sb)

        def _finish_rank(s, mvB, mvT_sb):
            rs = wpool.tile([128, 4], F32, name=f"rs_{s}", tag="rs", bufs=5)
            sgnscr = wpool.tile([128, 512], F32, name=f"sgn_{s}", tag="sgn", bufs=2)
            for j in range(4):
                nc.vector.tensor_scalar(sgnscr[:], mvB[:], mvT_sb[:, j:j + 1], None,
                                        AL.is_gt, AL.add, accum_out=rs[:, j:j + 1])
            # cnt_gt-256: bf16-exact integer in [-256,255]
            rsa = wpool.tile([128, 4], F32, name=f"rsa_{s}", tag="rsa", bufs=5)
            nc.vector.tensor_scalar(rsa[:], rs[:], -256.0, None, AL.add)
            rank_res[s] = rsa
            return rsa

        def finish_soft(s, mvB, mvT_sb):
            with _hpctx():
                expz = wpool.tile([1, 512], F32, name=f"expz_{s}", tag="expz", bufs=2)
                z_sb = wpool.tile([1, 1], F32, name=f"z_{s}", tag="z", bufs=4)
                nc.scalar.activation(expz[:], mvB[0:1, :], AF.Exp, accum_out=z_sb[:])
                z_ps = psm.tile([128, 4], F32, name=f"zps_{s}", tag="psT", bufs=1)
                nc.tensor.matmul(z_ps[:, 0:1],
                                 bass.AP(tensor=z_sb[:].tensor, offset=z_sb[:].offset,
                                         ap=[[z_sb[:].ap[0][0], 1], [0, 128]]),
                                 one_t, start=True, stop=True)
                rz128 = wpool.tile([128, 1], F32, name=f"rz128_{s}", tag="rz128", bufs=4)
                nc.vector.reciprocal(rz128[:], z_ps[:, 0:1])
                smc = wpool.tile([128, 4], BF16, name=f"smc_{s}", tag="smc", bufs=4)
                nc.scalar.activation(smc[:], mvT_sb[:], AF.Exp)
            soft_res[s] = (rz128, smc)

        def emit_wt(s):
            return _hp(_emit_wt, s)

        def _emit_wt(s):
            rsa = rank_res[s]
            wts = []
            for j in range(4):
                wt = wpool.tile([128, 512], BF16, name=f"wt_{s}_{j}", tag=f"wt{j}", bufs=2)
                nc.vector.tensor_scalar(wt[:], n2bB[:], rsa[:, j:j + 1], None, AL.is_equal)
                wts.append(wt)
            wt_res[s] = wts

        # ---- g (row-replicated via stride-0 smc columns) -> gmat -> cg4 ----
        def emit_gchain(s):
            rz, smc = soft_res[s]
            wts = wt_res[s]
            g_ps = psm.tile([128, 512], F32, name=f"gps_{s}", tag="psm", bufs=2)
            for j in range(4):
                nc.tensor.matmul(g_ps[:], _col0(smc[:, j:j + 1]), wts[j][:],
                                 start=(j == 0), stop=(j == 3))
            # gnB cols [128:640] = g; [0:128] = g[384:512] tail so the doubled-row
            # window [384,1024) of gmat is one contiguous write
            gnB = wpool.tile([128, 640], BF16, name=f"gnB_{s}", tag="gnB", bufs=2)
            nc.vector.tensor_scalar(gnB[:, 128:640], g_ps[:], rz[:], None, AL.mult)
            nc.vector.tensor_scalar(gnB[:, 0:128], g_ps[:, 384:512], rz[:], None, AL.mult)
            gmat = dpool.tile([128, 1024], BF16, name=f"gmat_{s}", tag="gmat", bufs=2)
            gd = gmat[:].tensor
            nc.scalar.dma_start(bass.AP(tensor=gd, offset=384, ap=[[1024, 128], [1, 640]]),
                                gnB[:])
            cg4 = wpool.tile([128, 512], BF16, name=f"cg4_{s}", tag="cg4", bufs=2)
            nc.scalar.dma_start(cg4[:], bass.AP(tensor=gd, offset=511, ap=[[1023, 128], [1, 512]]))
            chain_res[s] = cg4

        # ---- stage C: block-circulant matmul, 4 distinct stationary blocks ----
        def emit_stagec(s, vsb, split_out=False):
            cg4 = chain_res[s]
            o_sb = wpool.tile([128, 2048], BF16, name=f"osb_{s}", tag="osb", bufs=2)
            for tt in range(4):
                o_ps = pC.tile([128, 512], F32, name=f"ops_{s}_{tt}", tag="ops", bufs=2)
                for ss in range(4):
                    m = (tt - ss) % 4
                    nc.tensor.matmul(o_ps[:], cg4[:, 128 * m:128 * (m + 1)],
                                     vsb[:, 512 * ss:512 * (ss + 1)],
                                     start=(ss == 0), stop=(ss == 3))
                if tt % 2 == 0 or s >= 2:
                    nc.scalar.copy(o_sb[:, 512 * tt:512 * (tt + 1)], o_ps[:])
                else:
                    nc.vector.tensor_copy(o_sb[:, 512 * tt:512 * (tt + 1)], o_ps[:])
                if split_out:
                    nc.sync.dma_start(
                        bass.AP(tensor=out_all, offset=s * L * C + 128 * tt * C,
                                ap=[[C, 128], [1, C]]),
                        o_sb[:, 512 * tt:512 * (tt + 1)])
            if not split_out:
                nc.sync.dma_start(
                    bass.AP(tensor=out_all, offset=s * L * C,
                            ap=[[C, 128], [128 * C, 4], [1, C]]),
                    o_sb[:])

        rank_res, soft_res, wt_res, chain_res = {}, {}, {}, {}

        import contextlib

        def _nullhp():
            return contextlib.nullcontext()
        tc_high_priority_real = tc.high_priority
        USE_HP = True

        def _hpctx():
            return tc_high_priority_real() if USE_HP else contextlib.nullcontext()

        def _hp(fn, *a):
            with _hpctx():
                return fn(*a)

        # ================= emission schedule =================
        # Emission order IS the per-engine execution order. Queues: SP = bulk
        # loads (ring bufs=2 throttles them to just-in-time) + out stores;
        # ACT = PSUM->SBUF copies + DRAM writes; Pool = latency-critical DRAM
        # reads (its SEQ waits block nothing else); DVE = the rank/mask/g
        # elementwise chain.
        qk0sb = qpool.tile([128, 4096], BF16, name="qk0sb", tag="qkt", bufs=2)
        qksbs = [qpool.tile([128, 4096], BF16, name=f"qksb_{s}", tag="qkt", bufs=2)
                 for s in range(NB)]

        load_qk_half(qk0_all, 0, qk0sb, 0)
        nc.sync.dma_start(invc_t[:], invc_d.ap())
        load_qk_half(qk0_all, 0, qk0sb, 1)
        nc.sync.dma_start(small_t[:], small_d.ap())
        load_qk_half(qk_all, 0 * 1024 * L, qksbs[0], 0)
        load_qk_half(qk_all, 0 * 1024 * L, qksbs[0], 1)
        load_qk_half(qk_all, 1 * 1024 * L, qksbs[1], 0)
        load_qk_half(qk_all, 1 * 1024 * L, qksbs[1], 1)

        M0 = emit_M(qk0sb, "b0", ccs=(0, 1))
        emit_M(qk0sb, "b0", ccs=(2, 3), M_ps=M0)
        pd0 = emit_Pdiag_w(M0, "b0")
        R0 = emit_Pdiag_r(pd0, "b0")

        Ms = [None] * NB
        R_s = [None] * NB
        Ms[0] = emit_M(qksbs[0], "s0")
        R_s[0] = emit_Pdiag_r(emit_Pdiag_w(Ms[0], "s0"), "s0")
        load_qk_half(qk_all, 2 * 1024 * L, qksbs[2], 0)
        load_qk_half(qk_all, 2 * 1024 * L, qksbs[2], 1)

        Ms[1] = emit_M(qksbs[1], "s1")
        R_s[1] = emit_Pdiag_r(emit_Pdiag_w(Ms[1], "s1"), "s1")
        load_qk_half(qk_all, 3 * 1024 * L, qksbs[3], 0)
        load_qk_half(qk_all, 3 * 1024 * L, qksbs[3], 1)

        Ms[2] = emit_M(qksbs[2], "s2", ccs=(0, 1))
        mvB0 = emit_mv(R0, "b0")
        posmvT = emit_mvT(mvB0, "b0")
        rsa0 = finish_rank("b0", mvB0, posmvT)
        # batch-0 code row for REVERSED positions via anti-identity transposes
        n2bb_ps = psm.tile([1, 512], F32, name="n2bb_ps", bufs=1)
        for j in range(4):
            nc.tensor.matmul(n2bb_ps[0:1, 128 * (3 - j):128 * (4 - j)],
                             rsa0[:, j:j + 1], antiI_t, start=True, stop=True)
        n2bb_row = wpool.tile([1, 512], BF16, name="n2bb_row", bufs=1)
        nc.vector.tensor_copy(n2bb_row[:], n2bb_ps[:])
        n2bB_ps = psm.tile([128, 512], F32, name="n2bB_ps", tag="psm", bufs=2)
        nc.tensor.matmul(n2bB_ps[:], bass.AP(tensor=onebf_t.tensor, offset=onebf_t.offset,
                                             ap=[[onebf_t.ap[0][0], 1], [0, 128]]),
                         n2bb_row[:], start=True, stop=True)
        n2bB = wpool.tile([128, 512], BF16, name="n2bB", bufs=1)
        nc.vector.tensor_copy(n2bB[:], n2bB_ps[:])
        emit_M(qksbs[2], "s2", ccs=(2, 3), M_ps=Ms[2])
        R_s[2] = emit_Pdiag_r(emit_Pdiag_w(Ms[2], "s2"), "s2")

        Ms[3] = emit_M(qksbs[3], "s3", ccs=(0, 1))
        mvB_s0 = emit_mv(R_s[0], "s0")
        mvT_s0 = emit_mvT(mvB_s0, "s0")
        finish_rank(0, mvB_s0, mvT_s0)
        emit_wt(0)
        finish_soft(0, mvB_s0, mvT_s0)
        emit_M(qksbs[3], "s3", ccs=(2, 3), M_ps=Ms[3])
        R_s[3] = emit_Pdiag_r(emit_Pdiag_w(Ms[3], "s3"), "s3")
        vsbs = [load_v(0)]

        emit_gchain(0)
        mvB_s1 = emit_mv(R_s[1], "s1")
        mvT_s1 = emit_mvT(mvB_s1, "s1")
        finish_rank(1, mvB_s1, mvT_s1)
        emit_wt(1)
        finish_soft(1, mvB_s1, mvT_s1)
        vsbs.append(load_v(1))

        mvB_s2 = emit_mv(R_s[2], "s2")
        mvT_s2 = emit_mvT(mvB_s2, "s2")
        emit_gchain(1)
        finish_rank(2, mvB_s2, mvT_s2)
        emit_wt(2)
        finish_soft(2, mvB_s2, mvT_s2)
        vsbs.append(load_v(2))

        emit_stagec(0, vsbs[0])
        mvB_s3 = emit_mv(R_s[3], "s3")
        mvT_s3 = emit_mvT(mvB_s3, "s3")
        emit_gchain(2)
        finish_rank(3, mvB_s3, mvT_s3)
        emit_wt(3)
        finish_soft(3, mvB_s3, mvT_s3)
        vsbs.append(load_v(3))
        emit_stagec(1, vsbs[1])
        emit_gchain(3)
        emit_stagec(2, vsbs[2], split_out=True)
        emit_stagec(3, vsbs[3], split_out=True)

    nc.compile()
    _NC_CACHE = nc
    return nc


def kernel(queries, keys, values):
    q = np.ascontiguousarray(queries, dtype=np.float32).reshape(B, L, C)
    k = np.ascontiguousarray(keys, dtype=np.float32).reshape(B, L, C)
    v = np.ascontiguousarray(values, dtype=np.float32).reshape(B, L, C).astype(BF)
    # [B, C, L] -> [B, cchunk, {q,k}, 128, L]
    qT = np.ascontiguousarray(q.transpose(0, 2, 1)).astype(BF).reshape(B, 4, 128, L)
    kT = np.ascontiguousarray(k.transpose(0, 2, 1)).astype(BF).reshape(B, 4, 128, L)
    qk = np.stack([qT, kT], axis=2)  # [B, 4, 2, 128, L]
    nc = _build()
    in_maps = []
    for c in range(NCORES):
        sl = slice(NB * c, NB * (c + 1))
        in_maps.append({
            "qk_all": qk[sl],
            "qk0_all": qk[0],
            "v_all": v[sl],
        })
    res = run_bass_kernel_spmd(nc, in_maps, core_ids=list(range(NCORES)))
    out = np.concatenate([np.asarray(res.results[c]["out_all"]).astype(np.float32)
                          for c in range(NCORES)], axis=0)
    return out.reshape(B, L, H, E)


if __name__ == "__main__":
    rng = np.random.default_rng(0)
    qq = rng.standard_normal((B, L, H, E)).astype(np.float32)
    kk = rng.standard_normal((B, L, H, E)).astype(np.float32)
    vv = rng.standard_normal((B, L, H, E)).astype(np.float32)
    o = kernel(queries=qq, keys=kk, values=vv)
    print(o.shape, o.dtype, np.abs(o).max())
